# revision 2
# baseline (speedup 1.0000x reference)
"""Discounted cumsum along S via TensorE matmuls; tensor (8,16,4096,64), gamma (16,).

y[b,h,t,d] = gamma[h] * y[b,h,t-1,d] + x[b,h,t,d]

Strategy (8 cores, shard over B). Per core slab (16, 4096, 64) f32.
Layout: s-in-partition. Per h: 32 blocks j of 128 steps; SBUF tile
(128 parts = step-in-block, 32*64 free = (j, d)).

  y_j = L_h @ x_j + gamma^(t+1) * C_j[d]
  C_m[d] = sum_{j<m} gamma^{128(m-j)-1-u} x_j[u,d]   (all-pairs carries)

Heavy lifting on the (otherwise idle) TensorE as bf16 matmuls:
  - U: 32 mm/h, stationary Lb_h (128,128) = gamma^(t-u), moving xb_j (128,64)
  - C: 32 mm/h, stationary = sliding (128,32) slice of a Toeplitz pad
       Gpad_h (128,64) (cols k>=32 hold gamma^(127-u) * gamma^(128(k-32))),
       accumulated into one C psum tile (32,64)
Carry injection is fused with PSUM evacuation on DVE:
  yt = crep * gamma^(t+1) + U   (scalar_tensor_tensor, per-partition scalar)
where crep (128, 2048) is C replicated across partitions: C psum ->
Act copy (bf16) -> SBUF->SBUF re-partition DMA to one row (gpsimd queue)
-> gpsimd partition_broadcast.
DVE casts x to bf16. Queues: sync=x in + consts, scalar=y out,
gpsimd=carry row.
"""

import numpy as np

import concourse.bacc as bacc
import concourse.mybir as mybir
import concourse.tile as tile
from concourse.bass_utils import run_bass_kernel_spmd

F32 = mybir.dt.float32
BF16 = mybir.dt.bfloat16

B, H, S, D = 8, 16, 4096, 64
N_CORES = 8
P = 128           # steps per block
NJ = S // P       # 32 blocks per h
NQ = 4            # psum quarters per h
JQ = NJ // NQ     # 8 blocks per quarter
FREE = NJ * D     # 2048


def build_program():
    nc = bacc.Bacc("TRN2", target_bir_lowering=False, enable_partition_id=False)

    x_ext = nc.declare_dram_parameter("x", [H, S, D], F32, isOutput=False)
    lb_ext = nc.declare_dram_parameter("lb", [P, H * P], BF16, isOutput=False)
    gp_ext = nc.declare_dram_parameter("gp", [P, H * D], BF16, isOutput=False)
    gv_ext = nc.declare_dram_parameter("gv", [P, H], F32, isOutput=False)
    y_ext = nc.declare_dram_parameter("y", [H, S, D], F32, isOutput=True)

    with tile.TileContext(nc) as tc:
        with (
            tc.tile_pool(name="consts", bufs=1) as cp,
            tc.tile_pool(name="xf", bufs=4) as xfp,
            tc.tile_pool(name="xb", bufs=2) as xbp,
            tc.tile_pool(name="yt", bufs=8) as ytp,
            tc.tile_pool(name="cs", bufs=2) as csp,
            tc.tile_pool(name="crow", bufs=2) as crp,
            tc.tile_pool(name="up", bufs=4, space="PSUM") as upp,
            tc.tile_pool(name="cpp", bufs=2, space="PSUM") as cpp,
        ):
            lb = cp.tile([P, H * P], BF16)
            gp = cp.tile([P, H * D], BF16)
            gv = cp.tile([P, H], F32)
            nc.sync.dma_start(lb[:], lb_ext[:])
            nc.sync.dma_start(gp[:], gp_ext[:])
            nc.sync.dma_start(gv[:], gv_ext[:])

            xts, xbs = {}, {}

            def load(h):
                xv = x_ext[h].rearrange("(j p) d -> p j d", p=P)
                xt = xfp.tile([P, FREE], F32, tag="xt")
                nc.sync.dma_start(xt[:].rearrange("p (j d) -> p j d", d=D), xv)
                xts[h] = xt

            def conv(h):
                xb = xbp.tile([P, FREE], BF16, tag="xb")
                nc.vector.tensor_copy(xb[:], xts[h][:])
                xbs[h] = xb

            for h in range(H):
                load(h)
                conv(h)
                xb = xbs[h]
                lbh = lb[:, h * P:(h + 1) * P]
                gph = gp[:, h * D:(h + 1) * D]
                gvh = gv[:, h:h + 1]

                # C: all-pairs carries, accumulated over source blocks j
                cps = cpp.tile([NJ, D], F32, tag="cps")
                for j in range(NJ):
                    nc.tensor.matmul(
                        cps[:],
                        gph[:, 31 - j:63 - j],
                        xb[:, j * D:(j + 1) * D],
                        start=(j == 0), stop=(j == NJ - 1),
                    )
                # evac C (f32 psum -> bf16), re-partition, replicate
                cs = csp.tile([NJ, D], BF16, tag="cs")
                nc.scalar.copy(cs[:], cps[:])
                crow = crp.tile([1, FREE], BF16, tag="crow")
                nc.gpsimd.dma_start(crow[:], cs[:])
                crep = crp.tile([P, FREE], BF16, tag="crep")
                nc.gpsimd.partition_broadcast(crep[:], crow[:])

                # U matmuls (uniform closed groups); fused inject+evac on DVE
                for q in range(NQ):
                    uq = upp.tile([P, JQ * D], F32, tag="uq", name=f"u{h}_{q}")
                    for i in range(JQ):
                        j = q * JQ + i
                        nc.tensor.matmul(
                            uq[:, i * D:(i + 1) * D],
                            lbh,
                            xb[:, j * D:(j + 1) * D],
                            start=True, stop=True,
                        )
                    yt = ytp.tile([P, JQ * D], F32, tag="yt")
                    nc.vector.scalar_tensor_tensor(
                        out=yt[:],
                        in0=crep[:, q * JQ * D:(q + 1) * JQ * D],
                        scalar=gvh,
                        in1=uq[:],
                        op0=mybir.AluOpType.mult,
                        op1=mybir.AluOpType.add,
                    )
                    yv = y_ext[h].rearrange("(j p) d -> p j d", p=P)
                    nc.scalar.dma_start(
                        yv[:, q * JQ:(q + 1) * JQ, :],
                        yt[:].rearrange("p (j d) -> p j d", d=D))

    nc.finalize()
    return nc


def host_aux(gamma):
    """gamma-derived constants (host precompute, f64 internally)."""
    try:
        import ml_dtypes
        bf16 = ml_dtypes.bfloat16
    except ImportError:  # pragma: no cover
        import jax.numpy as jnp
        bf16 = jnp.bfloat16
    g = gamma.astype(np.float64)
    t = np.arange(P)
    with np.errstate(under="ignore", over="ignore"):
        # Lb[h][u, t'] = g^(t'-u), u <= t'
        du = t[None, :] - t[:, None]
        lb = np.where(du >= 0, g[:, None, None] ** np.maximum(du, 0), 0.0)
        # Gpad[h][u, k] = g^(127-u) * g^(128(k-32)) for k >= 32 else 0
        k = np.arange(D)
        a = g[:, None] ** (127 - t)[None, :]                          # (H, P)
        r = np.where(k[None, :] >= 32,
                     g[:, None] ** np.maximum(128.0 * (k - 32), 0.0)[None, :],
                     0.0)                                             # (H, D)
        gp = a[:, :, None] * r[:, None, :]                            # (H, P, D)
        # gvec[h][t'] = g^(t'+1), laid out (P, H) f32 per-partition scalar
        gv = g[:, None] ** (t + 1)[None, :]                           # (H, P)
    lb_t = np.ascontiguousarray(
        lb.transpose(1, 0, 2).reshape(P, H * P)).astype(bf16)
    gp_t = np.ascontiguousarray(
        gp.transpose(1, 0, 2).reshape(P, H * D)).astype(bf16)
    gv_t = np.ascontiguousarray(gv.T).astype(np.float32)
    return lb_t, gp_t, gv_t


_CACHE = {}


def kernel(tensor, gamma):
    tensor = np.asarray(tensor, dtype=np.float32)
    gamma = np.asarray(gamma, dtype=np.float32)
    assert tensor.shape == (B, H, S, D), tensor.shape

    if "nc" not in _CACHE:
        _CACHE["nc"] = build_program()
    nc = _CACHE["nc"]

    lb_t, gp_t, gv_t = host_aux(gamma)
    in_maps = [
        {"x": np.ascontiguousarray(tensor[b]), "lb": lb_t, "gp": gp_t,
         "gv": gv_t}
        for b in range(N_CORES)
    ]
    last_err = None
    for _attempt in range(3):
        try:
            res = run_bass_kernel_spmd(nc, in_maps, list(range(N_CORES)))
            break
        except Exception as e:  # transient NRT device wedge: retry
            last_err = e
    else:
        raise last_err
    out = np.stack([np.asarray(res.results[b]["y"]) for b in range(N_CORES)], axis=0)
    return out


# revision 3
# speedup vs baseline: 1.0113x; 1.0113x over previous
"""Discounted cumsum along S via TensorE matmuls; tensor (8,16,4096,64), gamma (16,).

y[b,h,t,d] = gamma[h] * y[b,h,t-1,d] + x[b,h,t,d]

Strategy (8 cores, shard over B). Per core slab (16, 4096, 64) f32.
Layout: s-in-partition. Per h: 32 blocks j of 128 steps; SBUF tile
(128 parts = step-in-block, 32*64 free = (j, d)).

  y_j = L_h @ x_j + gamma^(t+1) * C_j[d]
  C_m[d] = sum_{j<m} gamma^{128(m-j)-1-u} x_j[u,d]   (all-pairs carries)

Heavy lifting on the (otherwise idle) TensorE as bf16 matmuls:
  - U: 32 mm/h, stationary Lb_h (128,128) = gamma^(t-u), moving xb_j (128,64)
  - C: 32 mm/h, stationary = sliding (128,32) slice of a Toeplitz pad
       Gpad_h (128,64) (cols k>=32 hold gamma^(127-u) * gamma^(128(k-32))),
       accumulated into one C psum tile (32,64)
Carry injection is fused with PSUM evacuation on DVE:
  yt = crep * gamma^(t+1) + U   (scalar_tensor_tensor, per-partition scalar)
where crep (128, 2048) is C replicated across partitions: C psum ->
Act copy (bf16) -> SBUF->SBUF re-partition DMA to one row (gpsimd queue)
-> gpsimd partition_broadcast.
DVE casts x to bf16. Queues: sync=x in + consts, scalar=y out,
gpsimd=carry row.
"""

import numpy as np

import concourse.bacc as bacc
import concourse.mybir as mybir
import concourse.tile as tile
from concourse.bass_utils import run_bass_kernel_spmd

F32 = mybir.dt.float32
BF16 = mybir.dt.bfloat16

B, H, S, D = 8, 16, 4096, 64
N_CORES = 8
P = 128           # steps per block
NJ = S // P       # 32 blocks per h
NQ = 4            # psum quarters per h
JQ = NJ // NQ     # 8 blocks per quarter
FREE = NJ * D     # 2048


def build_program():
    nc = bacc.Bacc("TRN2", target_bir_lowering=False, enable_partition_id=False)

    x_ext = nc.declare_dram_parameter("x", [H, S, D], F32, isOutput=False)
    lb_ext = nc.declare_dram_parameter("lb", [P, H * P], BF16, isOutput=False)
    gp_ext = nc.declare_dram_parameter("gp", [P, H * D], BF16, isOutput=False)
    gv_ext = nc.declare_dram_parameter("gv", [P, H], F32, isOutput=False)
    y_ext = nc.declare_dram_parameter("y", [H, S, D], F32, isOutput=True)

    with tile.TileContext(nc) as tc:
        with (
            tc.tile_pool(name="consts", bufs=1) as cp,
            tc.tile_pool(name="xf", bufs=4) as xfp,
            tc.tile_pool(name="xb", bufs=2) as xbp,
            tc.tile_pool(name="yt", bufs=8) as ytp,
            tc.tile_pool(name="cs", bufs=2) as csp,
            tc.tile_pool(name="crow", bufs=2) as crp,
            tc.tile_pool(name="up", bufs=4, space="PSUM") as upp,
            tc.tile_pool(name="cpp", bufs=2, space="PSUM") as cpp,
        ):
            lb = cp.tile([P, H * P], BF16)
            gp = cp.tile([P, H * D], BF16)
            gv = cp.tile([P, H], F32)
            nc.sync.dma_start(lb[:], lb_ext[:])
            nc.sync.dma_start(gp[:], gp_ext[:])
            nc.sync.dma_start(gv[:], gv_ext[:])

            xts, xbs = {}, {}

            def load(h):
                xv = x_ext[h].rearrange("(j p) d -> p j d", p=P)
                xt = xfp.tile([P, FREE], F32, tag="xt")
                nc.sync.dma_start(xt[:].rearrange("p (j d) -> p j d", d=D), xv)
                xts[h] = xt

            def conv(h):
                xb = xbp.tile([P, FREE], BF16, tag="xb")
                nc.vector.tensor_copy(xb[:], xts[h][:])
                xbs[h] = xb

            creps = {}

            def carry(h):
                # C: all-pairs carries, accumulated over source blocks j
                xb = xbs[h]
                gph = gp[:, h * D:(h + 1) * D]
                cps = cpp.tile([NJ, D], F32, tag="cps")
                for j in range(NJ):
                    nc.tensor.matmul(
                        cps[:],
                        gph[:, 31 - j:63 - j],
                        xb[:, j * D:(j + 1) * D],
                        start=(j == 0), stop=(j == NJ - 1),
                    )
                # evac C (f32 psum -> bf16), re-partition, replicate
                cs = csp.tile([NJ, D], BF16, tag="cs")
                nc.scalar.copy(cs[:], cps[:])
                crow = crp.tile([1, FREE], BF16, tag="crow")
                nc.gpsimd.dma_start(crow[:], cs[:])
                crep = crp.tile([P, FREE], BF16, tag="crep")
                nc.gpsimd.partition_broadcast(crep[:], crow[:])
                creps[h] = crep

            # software pipeline: carry chain runs one head ahead so the Act
            # stream does evac(h+1) before out-triggers(h), and the DVE
            # stream does cast(h+1) before stt(h)
            load(0)
            conv(0)
            carry(0)
            load(1)

            for h in range(H):
                xb = xbs[h]
                lbh = lb[:, h * P:(h + 1) * P]
                gvh = gv[:, h:h + 1]
                crep = creps[h]

                if h + 1 < H:
                    conv(h + 1)
                    carry(h + 1)
                if h + 2 < H:
                    load(h + 2)

                # U matmuls (uniform closed groups); fused inject+evac on DVE
                for q in range(NQ):
                    uq = upp.tile([P, JQ * D], F32, tag="uq", name=f"u{h}_{q}")
                    for i in range(JQ):
                        j = q * JQ + i
                        nc.tensor.matmul(
                            uq[:, i * D:(i + 1) * D],
                            lbh,
                            xb[:, j * D:(j + 1) * D],
                            start=True, stop=True,
                        )
                    yt = ytp.tile([P, JQ * D], F32, tag="yt")
                    nc.vector.scalar_tensor_tensor(
                        out=yt[:],
                        in0=crep[:, q * JQ * D:(q + 1) * JQ * D],
                        scalar=gvh,
                        in1=uq[:],
                        op0=mybir.AluOpType.mult,
                        op1=mybir.AluOpType.add,
                    )
                    yv = y_ext[h].rearrange("(j p) d -> p j d", p=P)
                    nc.scalar.dma_start(
                        yv[:, q * JQ:(q + 1) * JQ, :],
                        yt[:].rearrange("p (j d) -> p j d", d=D))

    nc.finalize()
    return nc


def host_aux(gamma):
    """gamma-derived constants (host precompute, f64 internally)."""
    try:
        import ml_dtypes
        bf16 = ml_dtypes.bfloat16
    except ImportError:  # pragma: no cover
        import jax.numpy as jnp
        bf16 = jnp.bfloat16
    g = gamma.astype(np.float64)
    t = np.arange(P)
    with np.errstate(under="ignore", over="ignore"):
        # Lb[h][u, t'] = g^(t'-u), u <= t'
        du = t[None, :] - t[:, None]
        lb = np.where(du >= 0, g[:, None, None] ** np.maximum(du, 0), 0.0)
        # Gpad[h][u, k] = g^(127-u) * g^(128(k-32)) for k >= 32 else 0
        k = np.arange(D)
        a = g[:, None] ** (127 - t)[None, :]                          # (H, P)
        r = np.where(k[None, :] >= 32,
                     g[:, None] ** np.maximum(128.0 * (k - 32), 0.0)[None, :],
                     0.0)                                             # (H, D)
        gp = a[:, :, None] * r[:, None, :]                            # (H, P, D)
        # gvec[h][t'] = g^(t'+1), laid out (P, H) f32 per-partition scalar
        gv = g[:, None] ** (t + 1)[None, :]                           # (H, P)
    lb_t = np.ascontiguousarray(
        lb.transpose(1, 0, 2).reshape(P, H * P)).astype(bf16)
    gp_t = np.ascontiguousarray(
        gp.transpose(1, 0, 2).reshape(P, H * D)).astype(bf16)
    gv_t = np.ascontiguousarray(gv.T).astype(np.float32)
    return lb_t, gp_t, gv_t


_CACHE = {}


def kernel(tensor, gamma):
    tensor = np.asarray(tensor, dtype=np.float32)
    gamma = np.asarray(gamma, dtype=np.float32)
    assert tensor.shape == (B, H, S, D), tensor.shape

    if "nc" not in _CACHE:
        _CACHE["nc"] = build_program()
    nc = _CACHE["nc"]

    lb_t, gp_t, gv_t = host_aux(gamma)
    in_maps = [
        {"x": np.ascontiguousarray(tensor[b]), "lb": lb_t, "gp": gp_t,
         "gv": gv_t}
        for b in range(N_CORES)
    ]
    last_err = None
    for _attempt in range(3):
        try:
            res = run_bass_kernel_spmd(nc, in_maps, list(range(N_CORES)))
            break
        except Exception as e:  # transient NRT device wedge: retry
            last_err = e
    else:
        raise last_err
    out = np.stack([np.asarray(res.results[b]["y"]) for b in range(N_CORES)], axis=0)
    return out


# revision 4
# speedup vs baseline: 1.0175x; 1.0062x over previous
"""Discounted cumsum along S via TensorE matmuls; tensor (8,16,4096,64), gamma (16,).

y[b,h,t,d] = gamma[h] * y[b,h,t-1,d] + x[b,h,t,d]

Strategy (8 cores, shard over B). Per core slab (16, 4096, 64) f32.
Layout: s-in-partition. Per h: 32 blocks j of 128 steps; SBUF tile
(128 parts = step-in-block, 32*64 free = (j, d)).

  y_j = L_h @ x_j + gamma^(t+1) * C_j[d]
  C_m[d] = sum_{j<m} gamma^{128(m-j)-1-u} x_j[u,d]   (all-pairs carries)

Heavy lifting on the (otherwise idle) TensorE as bf16 matmuls:
  - U: 32 mm/h, stationary Lb_h (128,128) = gamma^(t-u), moving xb_j (128,64)
  - C: 32 mm/h, stationary = sliding (128,32) slice of a Toeplitz pad
       Gpad_h (128,64) (cols k>=32 hold gamma^(127-u) * gamma^(128(k-32))),
       accumulated into one C psum tile (32,64)
Carry injection is fused with PSUM evacuation on DVE:
  yt = crep * gamma^(t+1) + U   (scalar_tensor_tensor, per-partition scalar)
where crep (128, 2048) is C replicated across partitions: C psum ->
Act copy (bf16) -> SBUF->SBUF re-partition DMA to one row (gpsimd queue)
-> gpsimd partition_broadcast.
DVE casts x to bf16. Queues: sync=x in + consts, scalar=y out,
gpsimd=carry row.
"""

import numpy as np

import concourse.bacc as bacc
import concourse.mybir as mybir
import concourse.tile as tile
from concourse.bass_utils import run_bass_kernel_spmd

F32 = mybir.dt.float32
BF16 = mybir.dt.bfloat16

B, H, S, D = 8, 16, 4096, 64
N_CORES = 8
P = 128           # steps per block
NJ = S // P       # 32 blocks per h
NQ = 4            # psum quarters per h
JQ = NJ // NQ     # 8 blocks per quarter
FREE = NJ * D     # 2048


def build_program():
    nc = bacc.Bacc("TRN2", target_bir_lowering=False, enable_partition_id=False)

    x_ext = nc.declare_dram_parameter("x", [H, S, D], F32, isOutput=False)
    lb_ext = nc.declare_dram_parameter("lb", [P, H * P], BF16, isOutput=False)
    gp_ext = nc.declare_dram_parameter("gp", [P, H * D], BF16, isOutput=False)
    gv_ext = nc.declare_dram_parameter("gv", [P, H], F32, isOutput=False)
    y_ext = nc.declare_dram_parameter("y", [H, S, D], F32, isOutput=True)

    with tile.TileContext(nc) as tc:
        with (
            tc.tile_pool(name="consts", bufs=1) as cp,
            tc.tile_pool(name="xf", bufs=4) as xfp,
            tc.tile_pool(name="xb", bufs=3) as xbp,
            tc.tile_pool(name="yt", bufs=8) as ytp,
            tc.tile_pool(name="cs", bufs=3) as csp,
            tc.tile_pool(name="crow", bufs=3) as crp,
            tc.tile_pool(name="up", bufs=4, space="PSUM") as upp,
            tc.tile_pool(name="cpp", bufs=3, space="PSUM") as cpp,
        ):
            lb = cp.tile([P, H * P], BF16)
            gp = cp.tile([P, H * D], BF16)
            gv = cp.tile([P, H], F32)
            nc.sync.dma_start(lb[:], lb_ext[:])
            nc.sync.dma_start(gp[:], gp_ext[:])
            nc.sync.dma_start(gv[:], gv_ext[:])

            xts, xbs = {}, {}

            def load(h):
                xv = x_ext[h].rearrange("(j p) d -> p j d", p=P)
                xt = xfp.tile([P, FREE], F32, tag="xt")
                nc.sync.dma_start(xt[:].rearrange("p (j d) -> p j d", d=D), xv)
                xts[h] = xt

            def conv(h):
                xb = xbp.tile([P, FREE], BF16, tag="xb")
                nc.vector.tensor_copy(xb[:], xts[h][:])
                xbs[h] = xb

            creps = {}

            def carry(h):
                # C: all-pairs carries, accumulated over source blocks j
                xb = xbs[h]
                gph = gp[:, h * D:(h + 1) * D]
                cps = cpp.tile([NJ, D], F32, tag="cps")
                for j in range(NJ):
                    nc.tensor.matmul(
                        cps[:],
                        gph[:, 31 - j:63 - j],
                        xb[:, j * D:(j + 1) * D],
                        start=(j == 0), stop=(j == NJ - 1),
                    )
                # evac C (f32 psum -> bf16), re-partition, replicate
                cs = csp.tile([NJ, D], BF16, tag="cs")
                nc.scalar.copy(cs[:], cps[:])
                crow = crp.tile([1, FREE], BF16, tag="crow")
                nc.gpsimd.dma_start(crow[:], cs[:])
                crep = crp.tile([P, FREE], BF16, tag="crep")
                nc.gpsimd.partition_broadcast(crep[:], crow[:])
                creps[h] = crep

            # software pipeline: carry chain runs two heads ahead so the Act
            # stream does evac(h+1..2) before out-triggers(h), and the DVE
            # stream does cast(h+1..2) before stt(h)
            load(0)
            conv(0)
            carry(0)
            load(1)
            conv(1)
            carry(1)
            load(2)

            for h in range(H):
                xb = xbs[h]
                lbh = lb[:, h * P:(h + 1) * P]
                gvh = gv[:, h:h + 1]
                crep = creps[h]

                if h + 2 < H:
                    conv(h + 2)
                    carry(h + 2)
                if h + 3 < H:
                    load(h + 3)

                # U matmuls (uniform closed groups); fused inject+evac on DVE
                for q in range(NQ):
                    uq = upp.tile([P, JQ * D], F32, tag="uq", name=f"u{h}_{q}")
                    for i in range(JQ):
                        j = q * JQ + i
                        nc.tensor.matmul(
                            uq[:, i * D:(i + 1) * D],
                            lbh,
                            xb[:, j * D:(j + 1) * D],
                            start=True, stop=True,
                        )
                    yt = ytp.tile([P, JQ * D], F32, tag="yt")
                    nc.vector.scalar_tensor_tensor(
                        out=yt[:],
                        in0=crep[:, q * JQ * D:(q + 1) * JQ * D],
                        scalar=gvh,
                        in1=uq[:],
                        op0=mybir.AluOpType.mult,
                        op1=mybir.AluOpType.add,
                    )
                    yv = y_ext[h].rearrange("(j p) d -> p j d", p=P)
                    nc.scalar.dma_start(
                        yv[:, q * JQ:(q + 1) * JQ, :],
                        yt[:].rearrange("p (j d) -> p j d", d=D))

    nc.finalize()
    return nc


def host_aux(gamma):
    """gamma-derived constants (host precompute, f64 internally)."""
    try:
        import ml_dtypes
        bf16 = ml_dtypes.bfloat16
    except ImportError:  # pragma: no cover
        import jax.numpy as jnp
        bf16 = jnp.bfloat16
    g = gamma.astype(np.float64)
    t = np.arange(P)
    with np.errstate(under="ignore", over="ignore"):
        # Lb[h][u, t'] = g^(t'-u), u <= t'
        du = t[None, :] - t[:, None]
        lb = np.where(du >= 0, g[:, None, None] ** np.maximum(du, 0), 0.0)
        # Gpad[h][u, k] = g^(127-u) * g^(128(k-32)) for k >= 32 else 0
        k = np.arange(D)
        a = g[:, None] ** (127 - t)[None, :]                          # (H, P)
        r = np.where(k[None, :] >= 32,
                     g[:, None] ** np.maximum(128.0 * (k - 32), 0.0)[None, :],
                     0.0)                                             # (H, D)
        gp = a[:, :, None] * r[:, None, :]                            # (H, P, D)
        # gvec[h][t'] = g^(t'+1), laid out (P, H) f32 per-partition scalar
        gv = g[:, None] ** (t + 1)[None, :]                           # (H, P)
    lb_t = np.ascontiguousarray(
        lb.transpose(1, 0, 2).reshape(P, H * P)).astype(bf16)
    gp_t = np.ascontiguousarray(
        gp.transpose(1, 0, 2).reshape(P, H * D)).astype(bf16)
    gv_t = np.ascontiguousarray(gv.T).astype(np.float32)
    return lb_t, gp_t, gv_t


_CACHE = {}


def kernel(tensor, gamma):
    tensor = np.asarray(tensor, dtype=np.float32)
    gamma = np.asarray(gamma, dtype=np.float32)
    assert tensor.shape == (B, H, S, D), tensor.shape

    if "nc" not in _CACHE:
        _CACHE["nc"] = build_program()
    nc = _CACHE["nc"]

    lb_t, gp_t, gv_t = host_aux(gamma)
    in_maps = [
        {"x": np.ascontiguousarray(tensor[b]), "lb": lb_t, "gp": gp_t,
         "gv": gv_t}
        for b in range(N_CORES)
    ]
    last_err = None
    for _attempt in range(3):
        try:
            res = run_bass_kernel_spmd(nc, in_maps, list(range(N_CORES)))
            break
        except Exception as e:  # transient NRT device wedge: retry
            last_err = e
    else:
        raise last_err
    out = np.stack([np.asarray(res.results[b]["y"]) for b in range(N_CORES)], axis=0)
    return out
